# revision 1
# baseline (speedup 1.0000x reference)
"""Bidirectional cross-attention Trainium2 kernel (8-core SPMD).

Sharding: core = b*4 + hp  (b in {0,1} batches, hp in {0..3} head-pairs).
Each core handles 1 batch x 2 heads:
  - LayerNorm stats (bn_stats on DVE), LN-apply on GPSIMD (bf16)
  - DMA-xbar transposes y -> yT
  - QKV projections (bf16 matmuls), qkT/cqkT kept feature-major,
    v/cv kept row-major (with a ones-column for the column-softmax sums)
  - Per head: S = qk cqk^T on PE -> exp(SCALE*S) on ACT with fused row-sums Z
    -> E (bf16) ; E serves the context direction directly (contract over i);
    DMA-xbar transpose of E -> F serves the out direction (contract over j).
  - Unnormalized per-head output projections (fp32r matmuls), shipped fp16.
Host: divides by the softmax denominators (Z rows / W cols), sums the 4
head-pair partials per batch, adds biases.
"""

from contextlib import ExitStack

import numpy as np
import ml_dtypes

import concourse.bass as bass
from concourse import bacc
import concourse.tile as tile
import concourse.mybir as mybir
from concourse import bass_utils
from concourse.masks import make_identity

HEADS = 8
DIM_HEAD = 64
SCALE = DIM_HEAD ** -0.5
EPS = 1e-5
B = 2
N = 2048          # sequence length (both x and context)
DIM = 512
NCORES = 8
NT = N // 128     # 16 row tiles
KO = DIM // 128   # 4 contraction tiles

BF16 = mybir.dt.bfloat16
F32 = mybir.dt.float32
F32R = mybir.dt.float32r
FP16 = mybir.dt.float16

_nbf16 = ml_dtypes.bfloat16


def build_program(apply_bias: bool):
    nc = bacc.Bacc()
    AF = mybir.ActivationFunctionType
    ALU = mybir.AluOpType

    x_d = nc.dram_tensor("x", (NT, 128, DIM), BF16, kind="ExternalInput")
    c_d = nc.dram_tensor("ctx", (NT, 128, DIM), BF16, kind="ExternalInput")
    wqk_d = nc.dram_tensor("wqk", (KO, 128, 128), BF16, kind="ExternalInput")
    wcqk_d = nc.dram_tensor("wcqk", (KO, 128, 128), BF16, kind="ExternalInput")
    wv_d = nc.dram_tensor("wv", (KO, 128, 128), BF16, kind="ExternalInput")
    wcv_d = nc.dram_tensor("wcv", (KO, 128, 128), BF16, kind="ExternalInput")
    wout_d = nc.dram_tensor("wout", (128, DIM), F32, kind="ExternalInput")
    wcout_d = nc.dram_tensor("wcout", (128, DIM), F32, kind="ExternalInput")
    cvec_d = nc.dram_tensor("cvec", (1, 4 * 128), BF16, kind="ExternalInput")

    pout_d = nc.dram_tensor("pout", (2, 4, 128, N), FP16, kind="ExternalOutput")
    pcout_d = nc.dram_tensor("pcout", (2, 4, 128, N), FP16, kind="ExternalOutput")
    z_d = nc.dram_tensor("zsum", (2, 2, 128, NT), F32, kind="ExternalOutput")
    w_d = nc.dram_tensor("wsum", (2, 2, 1, N // 2), F32, kind="ExternalOutput")

    with tile.TileContext(nc) as tc:
        with ExitStack() as ctx:
            persist = ctx.enter_context(tc.tile_pool(name="persist", bufs=1))
            stage = ctx.enter_context(tc.tile_pool(name="stage", bufs=8))
            ypool = ctx.enter_context(tc.tile_pool(name="ypool", bufs=4))
            small = ctx.enter_context(tc.tile_pool(name="small", bufs=2))
            fpool = ctx.enter_context(tc.tile_pool(name="fpool", bufs=2))

            # ---- persistent SBUF tensors ----
            wqk = persist.tile([128, KO, 128], BF16, tag="wqk")
            wcqk = persist.tile([128, KO, 128], BF16, tag="wcqk")
            wv = persist.tile([128, KO, 128], BF16, tag="wv")
            wcv = persist.tile([128, KO, 128], BF16, tag="wcv")
            wout_raw = persist.tile([128, DIM], F32, tag="wout_raw")
            wcout_raw = persist.tile([128, DIM], F32, tag="wcout_raw")
            wout = persist.tile([128, DIM], F32R, tag="wout")
            wcout = persist.tile([128, DIM], F32R, tag="wcout")
            yT = persist.tile([128, KO, N], BF16, tag="yT")
            cT = persist.tile([128, KO, N], BF16, tag="cT")
            qkT = persist.tile([128, N], BF16, tag="qkT")
            cqkT = persist.tile([128, N], BF16, tag="cqkT")
            # row-major v / cv: per 128-row tile, per head, 66 cols
            # (64 data + col 64 = 1.0 for column sums + 1 pad)
            vrm = persist.tile([128, NT, 2, 66], BF16, tag="vrm")
            cvrm = persist.tile([128, NT, 2, 66], BF16, tag="cvrm")
            outT = persist.tile([128, N], F32R, tag="outT")
            coutT = persist.tile([128, N], F32R, tag="coutT")
            zfull = persist.tile([128, 2, 2, NT], F32, tag="zfull")

            nc.scalar.dma_start(wqk[:], wqk_d.rearrange("ko ki m -> ki ko m"))
            nc.scalar.dma_start(wcqk[:], wcqk_d.rearrange("ko ki m -> ki ko m"))
            nc.scalar.dma_start(wv[:], wv_d.rearrange("ko ki m -> ki ko m"))
            nc.scalar.dma_start(wcv[:], wcv_d.rearrange("ko ki m -> ki ko m"))
            nc.scalar.dma_start(wout_raw[:], wout_d[:, :])
            nc.scalar.dma_start(wcout_raw[:], wcout_d[:, :])
            nc.vector.tensor_copy(wout[:], wout_raw[:])
            nc.vector.tensor_copy(wcout[:], wcout_raw[:])

            if apply_bias:
                cvec = persist.tile([1, 4 * 128], BF16, tag="cvec")
                ones_row = persist.tile([1, 512], BF16, tag="ones_row")
                nc.scalar.dma_start(cvec[:], cvec_d[:, :])
                nc.vector.memset(ones_row[:], 1.0)

            nc.vector.memset(vrm[:, :, :, 65:66], 0.0)
            nc.vector.memset(cvrm[:, :, :, 65:66], 0.0)
            nc.vector.memset(vrm[:, :, :, 64:65], 1.0)
            nc.vector.memset(cvrm[:, :, :, 64:65], 1.0)

            epsc = persist.tile([128, 1], F32, tag="epsc")
            nc.vector.memset(epsc[:], EPS)

            ident = persist.tile([128, 128], BF16, tag="ident")
            make_identity(nc, ident[:])

            # ---- Phase 1: LayerNorm + transpose (groups of 4 row-tiles) ----
            with tc.tile_pool(name="ytpsum", bufs=4, space="PSUM") as ytpsum:
                for (src_d, dst_T) in ((x_d, yT), (c_d, cT)):
                    mvall = small.tile([128, NT, 2], F32, tag="mvall")
                    rstd = small.tile([128, NT], F32, tag="rstd")
                    nmr = small.tile([128, NT], F32, tag="nmr")
                    sd = small.tile([128, NT], F32, tag="sd")
                    for g in range(NT // 4):
                        gsl = slice(g * 4, g * 4 + 4)
                        xts = []
                        for t in range(g * 4, g * 4 + 4):
                            xt = stage.tile([128, DIM], BF16, tag="xt")
                            nc.scalar.dma_start(xt[:], src_d[t])
                            st6 = stage.tile([128, 6], F32, tag="st6")
                            nc.vector.bn_stats(st6[:], xt[:])
                            nc.vector.bn_aggr(mvall[:, t, :], st6[:, None, :])
                            xts.append(xt)
                        nc.scalar.activation(
                            sd[:, gsl], mvall[:, gsl, 1], AF.Sqrt, bias=epsc[:], scale=1.0
                        )
                        nc.vector.reciprocal(rstd[:, gsl], sd[:, gsl])
                        nc.vector.scalar_tensor_tensor(
                            nmr[:, gsl], rstd[:, gsl], -1.0, mvall[:, gsl, 0],
                            ALU.mult, ALU.mult,
                        )
                        for i, t in enumerate(range(g * 4, g * 4 + 4)):
                            yt = ypool.tile([128, DIM], BF16, tag="yt")
                            nc.vector.tensor_scalar(
                                yt[:], xts[i][:], rstd[:, t : t + 1], nmr[:, t : t + 1],
                                ALU.mult, ALU.add,
                            )
                            ytp = ytpsum.tile([128, KO, 128], BF16, tag="ytp")
                            for k in range(KO):
                                nc.tensor.transpose(
                                    ytp[:, k, :], yt[:, k * 128 : (k + 1) * 128],
                                    ident[:],
                                )
                            nc.scalar.copy(
                                dst_T[:, :, t * 128 : (t + 1) * 128], ytp[:]
                            )

            # ---- Phase 2: projections ----
            with tc.tile_pool(name="ppsum", bufs=2, space="PSUM") as ppsum, \
                 tc.tile_pool(name="vpsum", bufs=2, space="PSUM") as vpsum:
                for (proj_i, wT, srcT, dstT) in (
                    (0, wqk, yT, qkT), (1, wcqk, cT, cqkT)
                ):
                    for ic in range(4):
                        ps = ppsum.tile([128, 512], F32, tag="ppsum")
                        sl = slice(ic * 512, (ic + 1) * 512)
                        for k in range(KO):
                            nc.tensor.matmul(
                                ps[:], wT[:, k, :], srcT[:, k, sl],
                                start=(k == 0),
                                stop=(k == KO - 1 and not apply_bias),
                            )
                        if apply_bias:
                            nc.tensor.matmul(
                                ps[:], cvec[:, proj_i * 128 : (proj_i + 1) * 128],
                                ones_row[:, 0:512], start=False, stop=True,
                            )
                        nc.scalar.copy(dstT[:, sl], ps[:])
                for (proj_i, wT, srcT, dstRM) in (
                    (2, wv, yT, vrm), (3, wcv, cT, cvrm)
                ):
                    for t in range(NT):
                        ps = vpsum.tile([128, 128], F32, tag="vpsum")
                        sl = slice(t * 128, (t + 1) * 128)
                        for k in range(KO):
                            nc.tensor.matmul(
                                ps[:], srcT[:, k, sl], wT[:, k, :],
                                start=(k == 0),
                                stop=(k == KO - 1 and not apply_bias),
                            )
                        if apply_bias:
                            nc.tensor.matmul(
                                ps[:], ones_row[:, 0:128],
                                cvec[:, proj_i * 128 : (proj_i + 1) * 128],
                                start=False, stop=True,
                            )
                        nc.scalar.copy(
                            dstRM[:, t, :, 0:64],
                            ps[:].rearrange("p (h d) -> p h d", h=2),
                        )

            # ---- Phase 3: attention (one head at a time) + inline tails ----
            with tc.tile_pool(name="spsum", bufs=2, space="PSUM") as spsum, \
                 tc.tile_pool(name="cpsum", bufs=1, space="PSUM") as cpsum, \
                 tc.tile_pool(name="opsum", bufs=1, space="PSUM") as opsum, \
                 tc.tile_pool(name="tpsum", bufs=1, space="PSUM") as tpsum, \
                 tc.tile_pool(name="ostage", bufs=2) as ostage, \
                 tc.tile_pool(name="wtmpp", bufs=2) as wtmpp:
                for h in range(2):
                    hs = slice(h * 64, (h + 1) * 64)
                    E = persist.tile([128, NT, N], BF16, tag="E")
                    for t in range(NT):
                        for half in range(2):
                            sp = spsum.tile([128, 1024], F32, tag="spsum")
                            for jc in range(2):
                                j0 = half * 1024 + jc * 512
                                nc.tensor.matmul(
                                    sp[:, jc * 512 : (jc + 1) * 512],
                                    qkT[hs, t * 128 : (t + 1) * 128],
                                    cqkT[hs, j0 : j0 + 512],
                                    start=True, stop=True,
                                )
                            nc.scalar.activation(
                                E[:, t, half * 1024 : (half + 1) * 1024],
                                sp[:], AF.Exp, scale=SCALE,
                                accum_out=zfull[:, h, half, t : t + 1],
                            )
                    # context direction: coutT[d, j] += E[i,j] v[i,d] over i
                    for p in range(2):
                        cps = cpsum.tile([65, 1024], F32, tag="cpsum")
                        for t in range(NT):
                            for jc in range(2):
                                j0 = p * 1024 + jc * 512
                                nc.tensor.matmul(
                                    cps[:, jc * 512 : (jc + 1) * 512],
                                    vrm[:, t, h, 0:65],
                                    E[:, t, j0 : j0 + 512],
                                    start=(t == 0), stop=(t == NT - 1),
                                )
                        nc.vector.tensor_copy(
                            coutT[hs, p * 1024 : (p + 1) * 1024], cps[0:64, :]
                        )
                        wt = wtmpp.tile([1, 1024], F32, tag="wtmp")
                        nc.vector.tensor_copy(wt[:], cps[64:65, :])
                        nc.scalar.dma_start(w_d[h, p], wt[:])
                    # out direction via transposed E windows
                    for w in range(4):
                        F = fpool.tile([128, NT, 512], BF16, tag="F")
                        for tt in range(4):
                            t = w * 4 + tt
                            nc.sync.dma_start_transpose(
                                F[:, :, tt * 128 : (tt + 1) * 128], E[:, t, :]
                            )
                        ops = opsum.tile([64, 512], F32, tag="opsum")
                        for tj in range(NT):
                            nc.tensor.matmul(
                                ops[:], cvrm[:, tj, h, 0:64], F[:, tj, :],
                                start=(tj == 0), stop=(tj == NT - 1),
                            )
                        nc.vector.tensor_copy(
                            outT[hs, w * 512 : (w + 1) * 512], ops[:]
                        )
                    # inline tail for this head: unnormalized projections
                    for (srcT, wmat, dst_d) in (
                        (outT, wout, pout_d), (coutT, wcout, pcout_d)
                    ):
                        for m in range(4):
                            stg = ostage.tile([128, N], FP16, tag="ostage")
                            for w in range(4):
                                tp = tpsum.tile([128, 512], F32, tag="tpsum")
                                nc.tensor.matmul(
                                    tp[:],
                                    wmat[hs, m * 128 : (m + 1) * 128],
                                    srcT[hs, w * 512 : (w + 1) * 512],
                                    start=True, stop=True,
                                )
                                nc.vector.tensor_copy(
                                    stg[:, w * 512 : (w + 1) * 512], tp[:]
                                )
                            nc.scalar.dma_start(dst_d[h, m], stg[:])
                nc.scalar.dma_start(
                    z_d.rearrange("h l p t -> p h l t"), zfull[:]
                )

    nc.finalize()
    return nc


_cache = {}


def _get_program(apply_bias: bool):
    key = bool(apply_bias)
    if key not in _cache:
        _cache[key] = build_program(key)
    return _cache[key]


def make_in_maps(inputs):
    x = np.asarray(inputs["x"], np.float32)
    context = np.asarray(inputs["context"], np.float32)
    g_x = np.asarray(inputs["g_x"], np.float32)
    b_x = np.asarray(inputs["b_x"], np.float32)
    g_c = np.asarray(inputs["g_c"], np.float32)
    b_c = np.asarray(inputs["b_c"], np.float32)
    W_qk = np.asarray(inputs["W_qk"], np.float32)
    W_cqk = np.asarray(inputs["W_cqk"], np.float32)
    W_v = np.asarray(inputs["W_v"], np.float32)
    W_cv = np.asarray(inputs["W_cv"], np.float32)
    W_out = np.asarray(inputs["W_out"], np.float32)
    W_cout = np.asarray(inputs["W_cout"], np.float32)

    apply_bias = bool(np.any(b_x != 0) or np.any(b_c != 0))

    Wqk_g = g_x[:, None] * W_qk
    Wcqk_g = g_c[:, None] * W_cqk
    Wv_g = g_x[:, None] * W_v
    Wcv_g = g_c[:, None] * W_cv
    cq = b_x @ W_qk
    ccq = b_c @ W_cqk
    cvv = b_x @ W_v
    ccv = b_c @ W_cv

    xb = x.astype(_nbf16).reshape(B, NT, 128, DIM)
    cb = context.astype(_nbf16).reshape(B, NT, 128, DIM)

    in_maps = []
    for core in range(NCORES):
        b = core // 4
        hp = core % 4
        sl = slice(hp * 128, (hp + 1) * 128)
        cvec = np.concatenate([cq[sl], ccq[sl], cvv[sl], ccv[sl]]).astype(_nbf16)
        in_maps.append({
            "x": np.ascontiguousarray(xb[b]),
            "ctx": np.ascontiguousarray(cb[b]),
            "wqk": np.ascontiguousarray(
                Wqk_g[:, sl].astype(_nbf16).reshape(KO, 128, 128)),
            "wcqk": np.ascontiguousarray(
                Wcqk_g[:, sl].astype(_nbf16).reshape(KO, 128, 128)),
            "wv": np.ascontiguousarray(
                Wv_g[:, sl].astype(_nbf16).reshape(KO, 128, 128)),
            "wcv": np.ascontiguousarray(
                Wcv_g[:, sl].astype(_nbf16).reshape(KO, 128, 128)),
            "wout": np.ascontiguousarray(W_out[sl, :]),
            "wcout": np.ascontiguousarray(W_cout[sl, :]),
            "cvec": cvec.reshape(1, 512),
        })
    return in_maps, apply_bias


def assemble(results, inputs):
    b_out = np.asarray(inputs["b_out"], np.float32)
    b_cout = np.asarray(inputs["b_cout"], np.float32)
    out = np.zeros((B, N, DIM), np.float32)
    cout = np.zeros((B, N, DIM), np.float32)
    for core in range(NCORES):
        r = results[core]
        b = core // 4
        z = np.asarray(r["zsum"], np.float32).sum(1)  # [2,2,128,NT] -> [2,128,NT]
        wsum = np.asarray(r["wsum"], np.float32).reshape(2, N)
        pout = np.asarray(r["pout"], np.float32)     # [2, 4, 128, N]
        pcout = np.asarray(r["pcout"], np.float32)
        for h in range(2):
            zi = z[h].T.reshape(N)                   # z[h][p, t] -> i = t*128+p
            out[b] += (pout[h].reshape(DIM, N) / zi[None, :]).T
            cout[b] += (pcout[h].reshape(DIM, N) / wsum[h][None, :]).T
    out += b_out
    cout += b_cout
    return out, cout


def kernel(_trace=False, **inputs):
    in_maps, apply_bias = make_in_maps(inputs)
    nc = _get_program(apply_bias)
    res = bass_utils.run_bass_kernel_spmd(
        nc, in_maps, core_ids=list(range(NCORES)), trace=_trace,
    )
    out, cout = assemble(res.results, inputs)
    if _trace:
        return (out, cout), res
    return out, cout

